# revision 51
# baseline (speedup 1.0000x reference)
"""
Trainium2 Bass kernel for batched cross-attention:
  context[b] = softmax(q[b] @ tokens[b].T / sqrt(d)) @ tokens[b]
with x_latent (tokens) [16, 4096, 768] f32, prompts_latent (q) [16, 64, 768] f32.

Sharding: data-parallel over the batch dim - 16 batches / 8 cores = 2 per core.

Final design (72us vs the 93.2us ship-both-bf16 baseline, rel err 1.34e-2
vs the 2e-2 gate). The baseline was co-bound on DMA (16 engines ~75% busy
moving 24 MB/core at a hard ~22 GB/s/engine fabric limit) and PE (~62us
busy). Changes, in decreasing order of measured impact:
  - mm1's d-major copy of T ships in fp8 e3m4 (1.8% rms quantization)
    instead of bf16; q stays bf16. Score error ~0.018 absolute perturbs
    attention weights by ~1.8%, and since both the output and this error
    scale with sqrt(sum p^2), the output error is ~1.3e-2 relative absmax
    (measured e4m3 variant: 3.7e-2, scaling by 0.018/0.051) - under the
    gate with margin. mm2's n-major copy stays bf16 since its operand
    errors hit the output at full weight. 19.4 MB/core instead of 24.3.
  - Full prefetch: every load is issued up front from the SP sequencer
    ONLY (it runs no compute, so hwdge queue-full backpressure on issues
    never gates an exp; issuing loads from the ACT sequencer measured a
    ~20us exp delay). The whole working set (48 KB tt + 96 KB tn per
    partition) fits in SBUF, so the queue streams back-to-back and the PE
    is the only pacer (89% busy over its span).
  - Loads are batched 4 groups (one quad, 2048 tokens) per DMA in fully
    blocked HBM layouts (one contiguous descriptor per partition): the
    queue pays ~0.5-0.9us dead time per DMA instruction, so 12 big loads
    beat 36 small ones. The first quad is split finer for the head; both
    batches' qT ship in ONE up-front DMA (a qt load queued behind quad
    transfers cost a 3us batch-boundary stall).
  - The 4 PE transposes of each group's P chunks write slices of ONE psum
    bank, copied back by ONE DVE instruction (was: 4 banks + 4 copies;
    the bufs=2 WAR rotation stalled the PE 577ns per group).
  - The two final normalization halves are stored as each lands to
    shorten the exposed tail.
Rest as the baseline: mm1 S[64,512] per group accumulated over 6 d-chunks
(lhsT = qT chunk bf16 [128,64], rhs = tt chunk fp8 [128,512] - mixed
operand dtypes work), one wide exp [64,512] per group on ACT, row sums
via DVE reduce per group, mm2 [64,512|256] bf16 accumulation over 32
n-tiles, normalization by reciprocal-sum, natural [64,768] f32 output.
The group loop is software-pipelined two stages deep on the PE (mm1(g),
mm2(g-2), transposes(g-1)).

Measured dead ends (do not revisit): XBAR DmaTransposeAnt for either T
(2x per-byte DMA-engine cost + 3.2us/instr issue) or P (122us); gpsimd
software-DGE loads (98-105us); S^T/O^T full-partition matmul forms (PE
per-instr overhead + unhideable 128-col LDWEIGHTS dominate); fp8 e4m3
for mm1 (3.7e-2 rel err) or anything but bf16 for mm2; fp8 DoubleRow
(needs e4m3); per-group interleaved PSUM accumulators sharing a bank
with multiple start=True (whole-bank pending-zero wipes siblings).
"""

import os
import sys

import numpy as np

for _p in ("/opt/trn_rl_repo", "/root/.axon_site/_ro/trn_rl_repo"):
    if os.path.isdir(_p) and _p not in sys.path:
        sys.path.append(_p)

import ml_dtypes
from contextlib import ExitStack

import concourse.bass as bass
import concourse.mybir as mybir
import concourse.tile as tile
from concourse import bacc
from concourse.bass_utils import run_bass_kernel_spmd
from concourse.masks import make_identity

BF16 = ml_dtypes.bfloat16
FP8 = ml_dtypes.float8_e3m4

N_CORES = 8
B_TOTAL = 16
BPC = B_TOTAL // N_CORES  # batches per core
N = 4096  # tokens
D = 768   # latent dim
P = 64    # prompts
DC = D // 128   # d-chunks of 128
KC = D // 256   # d-chunks of 256 (DoubleRow contraction tiles)
NT = N // 128   # n-tiles of 128
G = N // 512    # groups of 512 tokens
SCALE = float(D) ** -0.5

_cached_nc = None


def build_bass_program() -> bass.Bass:
    nc = bacc.Bacc("TRN2", target_bir_lowering=False, debug=False)
    NQ = BPC * G // 4  # quads of 4 groups (2048 tokens), never cross batches
    qt = nc.declare_dram_parameter("qt", [128, BPC, DC, P], mybir.dt.bfloat16, isOutput=False)
    tt = nc.declare_dram_parameter("tt", [BPC, G // 4, 128, 4, DC, 512], mybir.dt.float8e3, isOutput=False)
    tn = nc.declare_dram_parameter("tn", [BPC, G // 4, 128, 4, 4, D], mybir.dt.bfloat16, isOutput=False)
    out = nc.declare_dram_parameter("out", [BPC, P, D], mybir.dt.float32, isOutput=True)

    with tile.TileContext(nc) as tc, ExitStack() as ctx:
        singles = ctx.enter_context(tc.tile_pool(name="singles", bufs=1))
        qt_pool = ctx.enter_context(tc.tile_pool(name="qt", bufs=2))
        tt_pool = ctx.enter_context(tc.tile_pool(name="ttg", bufs=4))
        tn_pool = ctx.enter_context(tc.tile_pool(name="tnt", bufs=4))
        p_pool = ctx.enter_context(tc.tile_pool(name="pexp", bufs=4))
        pt_pool = ctx.enter_context(tc.tile_pool(name="ptT", bufs=4))
        sums_pool = ctx.enter_context(tc.tile_pool(name="sums", bufs=2))
        o_pool = ctx.enter_context(tc.tile_pool(name="osb", bufs=2))

        psum_s = ctx.enter_context(tc.tile_pool(name="psum_s", bufs=4, space="PSUM"))
        psum_pt = ctx.enter_context(tc.tile_pool(name="psum_pt", bufs=2, space="PSUM"))
        psum_o = ctx.enter_context(tc.tile_pool(name="psum_o", bufs=1, space="PSUM"))

        ident = singles.tile([P, P], mybir.dt.bfloat16)
        make_identity(nc, ident)

        # Per-batch state; o accumulators allocated lazily at first mm2 so
        # batch 1's PSUM allocation doesn't wait on batch 0's release.
        qt_ts = [None] * BPC
        sums_t = [None] * BPC
        o_ab = [None] * BPC

        def transpose_stage(p_sb, b, g):
            # PE transposes of the 4 P chunks into slices of ONE psum bank
            # (each transpose's start only re-flags already-written regions
            # pending-zero, which is harmless for reads), then a single DVE
            # copy to SBUF. One copy + one WAR rotation per group instead of
            # four of each.
            pt_ps = psum_pt.tile([128, 4, P], mybir.dt.bfloat16)
            for j in range(4):
                nc.tensor.transpose(pt_ps[:, j, :], p_sb[:, j * 128:(j + 1) * 128], ident)
            pt_sb = pt_pool.tile([128, 4, P], mybir.dt.bfloat16)
            nc.vector.tensor_copy(pt_sb, pt_ps)
            return [pt_sb[:, j, :] for j in range(4)]

        def mm2_stage(pt_sbs, tn_g, b, g):
            if o_ab[b] is None:
                o_a = psum_o.tile([P, 512], mybir.dt.float32, tag="o_a")
                o_b_ = psum_o.tile([P, 256], mybir.dt.float32, tag="o_b")
                o_ab[b] = (o_a, o_b_)
            o_a, o_b_ = o_ab[b]
            for j in range(4):
                nt = g * 4 + j
                nc.tensor.matmul(
                    o_a,
                    lhsT=pt_sbs[j],
                    rhs=tn_g[:, j, 0:512],
                    start=(nt == 0),
                    stop=(nt == NT - 1),
                )
                nc.tensor.matmul(
                    o_b_,
                    lhsT=pt_sbs[j],
                    rhs=tn_g[:, j, 512:768],
                    start=(nt == 0),
                    stop=(nt == NT - 1),
                )
            if g == G - 1:
                finish_batch(b)

        def finish_batch(b):
            # normalization + store; emitted immediately after the batch's
            # last mm2 so its PSUM accumulators release quickly. Each half
            # is stored as soon as its normalization lands to shorten the
            # exposed tail after the last matmul.
            tot = sums_pool.tile([P, 1], mybir.dt.float32)
            nc.vector.reduce_sum(tot, sums_t[b], axis=mybir.AxisListType.X)
            rec = sums_pool.tile([P, 1], mybir.dt.float32)
            nc.vector.reciprocal(rec, tot)
            o_a, o_b_ = o_ab[b]
            o_sb = o_pool.tile([P, D], mybir.dt.float32)
            nc.vector.tensor_scalar_mul(o_sb[:, 512:768], o_b_, rec)
            nc.sync.dma_start(out=out[b][:, 512:768], in_=o_sb[:, 512:768])
            nc.vector.tensor_scalar_mul(o_sb[:, 0:512], o_a, rec)
            nc.sync.dma_start(out=out[b][:, 0:512], in_=o_sb[:, 0:512])

        # Full prefetch: issue EVERY load up front so no DMA issue ever sits
        # behind a compute-waiting instruction in a sequencer's in-order
        # stream. All issues go on the SP (sync) sequencer, which runs no
        # compute, so queue-full backpressure never delays an exp. Loads are
        # batched four groups (one quad) per DMA - the queue pays ~0.5-0.9us
        # of dead time per DMA instruction, so 12 big loads beat 36 small
        # ones. The whole working set (48 KB tt + 96 KB tn per partition)
        # fits in SBUF; the first quad is split finer so the head of the
        # pipeline isn't gated on a 24 KB/partition burst.
        qt_both = qt_pool.tile([128, BPC, DC, P], mybir.dt.bfloat16, name="qt_both")
        nc.sync.dma_start(out=qt_both, in_=qt[:])
        for b in range(BPC):
            qt_ts[b] = qt_both[:, b]
        tt_quads = [None] * NQ
        tn_quads = [None] * NQ
        for q in range(NQ):
            b, qq = divmod(q, G // 4)
            tt_q = tt_pool.tile([128, 4, DC, 512], mybir.dt.float8e3, name="tt_q")
            tn_q = tn_pool.tile([128, 4, 4, D], mybir.dt.bfloat16, name="tn_q")
            if q == 0:
                for c in range(DC):
                    nc.sync.dma_start(out=tt_q[:, 0, c], in_=tt[b, qq][:, 0, c])
                nc.sync.dma_start(out=tn_q[:, 0], in_=tn[b, qq][:, 0])
                nc.sync.dma_start(out=tt_q[:, 1:4], in_=tt[b, qq][:, 1:4])
                nc.sync.dma_start(out=tn_q[:, 1:4], in_=tn[b, qq][:, 1:4])
            else:
                nc.sync.dma_start(out=tt_q, in_=tt[b, qq])
                nc.sync.dma_start(out=tn_q, in_=tn[b, qq])
            tt_quads[q] = tt_q
            tn_quads[q] = tn_q

        # One continuous two-stage software pipeline across BOTH batches:
        # PE program order per iteration is [mm1(i)] [mm2(i-2)]
        # [transposes(i-1)] - no pipeline flush at the batch boundary.
        tr_q = []   # (p_sb, tn_g, b, g) awaiting transpose stage (depth 2)
        mm2_q = []  # (pt_sbs, tn_g, b, g) awaiting mm2 stage
        for idx in range(BPC * G):
            b, g = divmod(idx, G)
            if g == 0:
                sums_t[b] = sums_pool.tile([P, G], mybir.dt.float32, tag="sums", name="sums")
            qt_t = qt_ts[b]
            tt_g = tt_quads[idx // 4][:, idx % 4]
            tn_g = tn_quads[idx // 4][:, idx % 4]

            s_ps = psum_s.tile([P, 512], mybir.dt.float32)
            for c in range(DC):
                nc.tensor.matmul(
                    s_ps,
                    lhsT=qt_t[:, c],
                    rhs=tt_g[:, c],
                    start=(c == 0),
                    stop=(c == DC - 1),
                )

            # P = exp(S * scale), cast to bf16, in one wide ACT instruction
            # (the transposes consume it a full pipeline stage later, so
            # chunking buys nothing and the ACT sequencer stays light).
            # Row sums on DVE.
            p_sb = p_pool.tile([P, 512], mybir.dt.bfloat16)
            nc.scalar.activation(
                out=p_sb,
                in_=s_ps,
                func=mybir.ActivationFunctionType.Exp,
                scale=SCALE,
            )
            nc.vector.reduce_sum(
                sums_t[b][:, g:g + 1], p_sb, axis=mybir.AxisListType.X
            )

            if len(tr_q) == 2:
                if mm2_q:
                    mm2_stage(*mm2_q.pop(0))
                p_sb0, tn_g0, b0, g0 = tr_q.pop(0)
                pt_sbs = transpose_stage(p_sb0, b0, g0)
                mm2_q.append((pt_sbs, tn_g0, b0, g0))
            tr_q.append((p_sb, tn_g, b, g))
        while tr_q:
            if mm2_q:
                mm2_stage(*mm2_q.pop(0))
            p_sb0, tn_g0, b0, g0 = tr_q.pop(0)
            pt_sbs = transpose_stage(p_sb0, b0, g0)
            mm2_q.append((pt_sbs, tn_g0, b0, g0))
        while mm2_q:
            mm2_stage(*mm2_q.pop(0))

    nc.compile()
    return nc


def _get_nc() -> bass.Bass:
    global _cached_nc
    if _cached_nc is None:
        _cached_nc = build_bass_program()
    return _cached_nc


def _make_in_maps(x_latent: np.ndarray, prompts_latent: np.ndarray):
    # tt_h[b, q, p, h, c, nn] = T[b, (4q+h)*512 + nn, c*128 + p]
    tt_h = np.ascontiguousarray(
        x_latent.astype(FP8).reshape(B_TOTAL, G // 4, 4, 512, DC, 128).transpose(0, 1, 5, 2, 4, 3)
    )
    # tn_h[b, q, p, h, t, d] = T[b, (4q+h)*512 + t*128 + p, d]
    tn_h = np.ascontiguousarray(
        x_latent.astype(BF16).reshape(B_TOTAL, G // 4, 4, 4, 128, D).transpose(0, 1, 4, 2, 3, 5)
    )
    # qt_h[b, p, c, z] = Q[b, z, c*128 + p], then per-core to [p, b, c, z]
    qt_h = np.ascontiguousarray(
        prompts_latent.astype(BF16).reshape(B_TOTAL, P, DC, 128).transpose(0, 3, 2, 1)
    )
    return [
        {
            "qt": np.ascontiguousarray(
                qt_h[c * BPC:(c + 1) * BPC].transpose(1, 0, 2, 3)
            ),
            "tt": tt_h[c * BPC:(c + 1) * BPC],
            "tn": tn_h[c * BPC:(c + 1) * BPC],
        }
        for c in range(N_CORES)
    ]


def run(x_latent: np.ndarray, prompts_latent: np.ndarray, trace: bool = False):
    """Run on all 8 cores; returns (output [16, 64, 768] f32, BassKernelResults)."""
    nc = _get_nc()
    in_maps = _make_in_maps(np.asarray(x_latent), np.asarray(prompts_latent))
    res = run_bass_kernel_spmd(nc, in_maps, list(range(N_CORES)), trace=trace)
    out = np.concatenate([np.asarray(r["out"]) for r in res.results], axis=0)
    return out.astype(np.float32), res


def kernel(x_latent: np.ndarray, prompts_latent: np.ndarray) -> np.ndarray:
    out, _ = run(x_latent, prompts_latent, trace=False)
    return out
